# revision 9
# baseline (speedup 1.0000x reference)
"""Trainium2 Bass kernel for nn_Attention_Dec (dense cross-attention, B=2 N=2048
DIM=1024 H=16 heads of d=64, padding mask, softmax over x-positions).

Sharding: core c handles batch b=c//4 and 4 heads h0=(c%4)*4 (tensor-parallel
over heads within a batch).  Each core computes a partial output
Y_c = (softmax-attn for its 4 heads) @ W_out[:, cols].T  of shape [2048, 1024];
host sums the 4 partials per batch, adds b_out, and writes NaN rows where the
(front-padded) mask is False — matching the reference's all--inf softmax NaNs.

Device-side per core (all matmuls float32r, fp32 PSUM accumulate):
  KT[d,i] = Wk_h @ tar^T          (i = tar positions)
  QT[d,j] = (Wq_h/32) @ x^T       (j = x positions; scale folded into weights)
  V[j,d]  = x @ Wv_h^T            (+ ones column for the softmax denominator)
  S^T[j,i] = QT^T·KT  (per head, K=64 contraction)
  P^T = exp(S^T + maskadd_j)      (per-partition ACT bias masks x positions;
                                   -30000 underflows exp to exactly 0)
  OT[d,i] = sum_j V_aug[j,d]·P^T[j,i]  (row 64 = softmax denominator den_i)
  AT[d,i] = OT[d,i] * (1/den_i)   (batched reciprocal + ones-broadcast matmul)
  Y[n,o] += AT_pair[:,n]^T @ WoT_pair  (K=128, accumulated over head pairs)

PSUM layout: tag "s2" = 2x [128,1024] (4 banks, double-buffered: projections,
S^T, recip-broadcast, final Y) + tag "ot" = 1x [65,2048] (4 banks: V rounds,
then the per-head PV accumulator).
"""

from contextlib import ExitStack

import numpy as np

B, N, DIM, H = 2, 2048, 1024, 16
D = 64
HPC = 4  # heads per core
NCORES = 8
NJC = N // 128  # 16 j-chunks
NKC = DIM // 128  # 8 contraction chunks

_CACHE = {}


def _build_nc():
    import concourse.tile as tile
    from concourse import bacc, mybir

    F32 = mybir.dt.float32
    F32R = mybir.dt.float32r
    EXP = mybir.ActivationFunctionType.Exp

    nc = bacc.Bacc("TRN2", debug=False, num_devices=NCORES)

    xT = nc.dram_tensor("xT", [DIM, N], F32R, kind="ExternalInput").ap()
    tarT = nc.dram_tensor("tarT", [DIM, N], F32R, kind="ExternalInput").ap()
    wqT = nc.dram_tensor("wqT", [DIM, HPC * D], F32R, kind="ExternalInput").ap()
    wvT = nc.dram_tensor("wvT", [DIM, HPC * D], F32R, kind="ExternalInput").ap()
    wkT = nc.dram_tensor("wkT", [DIM, HPC * D], F32R, kind="ExternalInput").ap()
    woTp = nc.dram_tensor("woTp", [2, 128, DIM], F32R, kind="ExternalInput").ap()
    maskadd = nc.dram_tensor("maskadd", [128, NJC], F32, kind="ExternalInput").ap()
    onesc = nc.dram_tensor("onesc", [128, HPC], F32R, kind="ExternalInput").ap()
    onesq = nc.dram_tensor("onesq", [128, D], F32R, kind="ExternalInput").ap()
    y = nc.dram_tensor("y", [N, DIM], F32, kind="ExternalOutput").ap()

    with tile.TileContext(nc) as tc, ExitStack() as ctx:
        consts = ctx.enter_context(tc.tile_pool(name="consts", bufs=1))
        wpool = ctx.enter_context(tc.tile_pool(name="wpool", bufs=1))
        qkv = ctx.enter_context(tc.tile_pool(name="qkv", bufs=1))
        ps2 = ctx.enter_context(tc.tile_pool(name="ps2", bufs=2, space="PSUM"))
        pot = ctx.enter_context(tc.tile_pool(name="pot", bufs=1, space="PSUM"))

        mk = consts.tile([128, NJC], F32, name="mk")
        nc.sync.dma_start(mk[:], maskadd[:])
        ones_c = consts.tile([128, HPC], F32R, name="ones_c")
        nc.sync.dma_start(ones_c[:], onesc[:])
        ones_q = consts.tile([128, D], F32R, name="ones_q")
        nc.sync.dma_start(ones_q[:], onesq[:])

        wq_t, wv_t, wk_t = [], [], []
        for kc in range(NKC):
            t = wpool.tile([128, HPC * D], F32R, name=f"wq{kc}", tag=f"wq{kc}")
            nc.sync.dma_start(t[:], wqT[kc * 128 : (kc + 1) * 128, :])
            wq_t.append(t)
            t = wpool.tile([128, HPC * D], F32R, name=f"wv{kc}", tag=f"wv{kc}")
            nc.sync.dma_start(t[:], wvT[kc * 128 : (kc + 1) * 128, :])
            wv_t.append(t)
            t = wpool.tile([128, HPC * D], F32R, name=f"wk{kc}", tag=f"wk{kc}")
            nc.sync.dma_start(t[:], wkT[kc * 128 : (kc + 1) * 128, :])
            wk_t.append(t)

        KT = [qkv.tile([128, N], F32R, name=f"KT{p}", tag=f"KT{p}") for p in range(2)]
        QT = [qkv.tile([128, N], F32R, name=f"QT{p}", tag=f"QT{p}") for p in range(2)]
        VA = [
            qkv.tile([128, HPC, D + 1], F32R, name=f"VA{jc}", tag=f"VA{jc}")
            for jc in range(NJC)
        ]

        # ---- projections: KT from tarT (pool released), then QT+V from xT ----
        with tc.tile_pool(name="tart", bufs=1) as tartp:
            tt = []
            for kc in range(NKC):
                t = tartp.tile([128, N], F32R, name=f"tart{kc}", tag=f"tart{kc}")
                nc.sync.dma_start(t[:], tarT[kc * 128 : (kc + 1) * 128, :])
                tt.append(t)
            for p in range(2):
                for hf in range(2):
                    ps = ps2.tile(
                        [128, N // 2], F32, name=f"psk{p}{hf}",
                        tag="sA" if (2 * p + hf) % 2 == 0 else "sB", bufs=1,
                    )
                    for ib in range(2):
                        off = hf * 1024 + ib * 512
                        for kc in range(NKC):
                            nc.tensor.matmul(
                                ps[:, ib * 512 : (ib + 1) * 512],
                                wk_t[kc][:, p * 128 : (p + 1) * 128],
                                tt[kc][:, off : off + 512],
                                start=(kc == 0),
                                stop=(kc == NKC - 1),
                            )
                    nc.scalar.copy(KT[p][:, hf * 1024 : (hf + 1) * 1024], ps[:])

        with tc.tile_pool(name="xt", bufs=1) as xtp:
            xt = []
            for kc in range(NKC):
                t = xtp.tile([128, N], F32R, name=f"xt{kc}", tag=f"xt{kc}")
                nc.sync.dma_start(t[:], xT[kc * 128 : (kc + 1) * 128, :])
                xt.append(t)
            for p in range(2):
                for hf in range(2):
                    ps = ps2.tile(
                        [128, N // 2], F32, name=f"psq{p}{hf}",
                        tag="sA" if (2 * p + hf) % 2 == 0 else "sB", bufs=1,
                    )
                    for ib in range(2):
                        off = hf * 1024 + ib * 512
                        for kc in range(NKC):
                            nc.tensor.matmul(
                                ps[:, ib * 512 : (ib + 1) * 512],
                                wq_t[kc][:, p * 128 : (p + 1) * 128],
                                xt[kc][:, off : off + 512],
                                start=(kc == 0),
                                stop=(kc == NKC - 1),
                            )
                    nc.scalar.copy(QT[p][:, hf * 1024 : (hf + 1) * 1024], ps[:])
            for jc in range(NJC):
                pv = ps2.tile(
                    [128, HPC * D], F32, name=f"pv{jc}",
                    tag="sA" if jc % 2 == 0 else "sB", bufs=1,
                )
                for kc in range(NKC):
                    nc.tensor.matmul(
                        pv[:],
                        xt[kc][:, jc * 128 : (jc + 1) * 128],
                        wv_t[kc][:],
                        start=(kc == 0),
                        stop=(kc == NKC - 1),
                    )
                nc.vector.tensor_copy(VA[jc][:, :, 0:D], pv[:])
                nc.vector.tensor_copy(VA[jc][:, :, D], ones_c[:])

        # ---- late pool (reuses released tart/xt space) ----
        late = ctx.enter_context(tc.tile_pool(name="late", bufs=1))
        wo_t = []
        for p in range(2):
            t = late.tile([128, DIM], F32R, name=f"wo{p}", tag=f"wo{p}")
            nc.sync.dma_start(t[:], woTp[p])
            wo_t.append(t)
        OTs = [
            late.tile([D, N], F32, name=f"OTs{h}", tag=f"OTs{h}") for h in range(HPC)
        ]
        AT = [late.tile([128, N], F32R, name=f"AT{p}", tag=f"AT{p}") for p in range(2)]
        den_t = late.tile([128, N], F32, name="den_t")
        rdq = late.tile([128, N], F32R, name="rdq")
        ptp = ctx.enter_context(tc.tile_pool(name="ptp", bufs=4))
        ysb = ctx.enter_context(tc.tile_pool(name="ysb", bufs=2))

        # ---- attention: head pairs row-tiled on the PE (A at rows 0-63,
        # B at rows 64-127), i-halves outer so S_A+S_B+OT_A+OT_B = 8 banks ----
        for p in range(2):
            hA, hB = 2 * p, 2 * p + 1
            for ih in range(2):
                ioff = ih * 1024
                otA = pot.tile([D + 1, N // 2], F32, name=f"otA{p}_{ih}", tag="otA")
                otB = pot.tile([D + 1, N // 2], F32, name=f"otB{p}_{ih}", tag="otB")
                for jc in range(NJC):
                    stA = ps2.tile(
                        [128, N // 2], F32, name=f"stA{p}_{jc}_{ih}", tag="sA",
                        bufs=1,
                    )
                    stB = ps2.tile(
                        [128, N // 2], F32, name=f"stB{p}_{jc}_{ih}", tag="sB",
                        bufs=1,
                    )
                    for ib in range(2):
                        off = ioff + ib * 512
                        nc.tensor.matmul(
                            stA[:, ib * 512 : (ib + 1) * 512],
                            QT[p][0:D, jc * 128 : (jc + 1) * 128],
                            KT[p][0:D, off : off + 512],
                            start=True,
                            stop=True,
                        )
                        nc.tensor.matmul(
                            stB[:, ib * 512 : (ib + 1) * 512],
                            QT[p][D:128, jc * 128 : (jc + 1) * 128],
                            KT[p][D:128, off : off + 512],
                            start=True,
                            stop=True,
                        )
                    ptA = ptp.tile(
                        [128, N // 2], F32R, name=f"ptA{p}_{jc}_{ih}", tag="pt"
                    )
                    nc.scalar.activation(
                        ptA[:], stA[:], EXP, bias=mk[:, jc : jc + 1], scale=1.0
                    )
                    ptB = ptp.tile(
                        [128, N // 2], F32R, name=f"ptB{p}_{jc}_{ih}", tag="pt"
                    )
                    nc.scalar.activation(
                        ptB[:], stB[:], EXP, bias=mk[:, jc : jc + 1], scale=1.0
                    )
                    for ib in range(2):
                        nc.tensor.matmul(
                            otA[:, ib * 512 : (ib + 1) * 512],
                            VA[jc][:, hA, :],
                            ptA[:, ib * 512 : (ib + 1) * 512],
                            start=(jc == 0),
                            stop=(jc == NJC - 1),
                        )
                        nc.tensor.matmul(
                            otB[:, ib * 512 : (ib + 1) * 512],
                            VA[jc][:, hB, :],
                            ptB[:, ib * 512 : (ib + 1) * 512],
                            start=(jc == 0),
                            stop=(jc == NJC - 1),
                        )
                nc.vector.tensor_copy(
                    OTs[hA][:, ioff : ioff + 1024], otA[0:D, :]
                )
                nc.scalar.copy(
                    den_t[32 * hA : 32 * hA + 1, ioff : ioff + 1024], otA[D : D + 1, :]
                )
                nc.vector.tensor_copy(
                    OTs[hB][:, ioff : ioff + 1024], otB[0:D, :]
                )
                nc.scalar.copy(
                    den_t[32 * hB : 32 * hB + 1, ioff : ioff + 1024], otB[D : D + 1, :]
                )

        # ---- normalize: AT[pair] rows = OTs_h * broadcast(1/den_h) ----
        with nc.allow_low_precision(reason="f32r reciprocal, 1e-4 rel is fine"):
            nc.vector.reciprocal(rdq[0 : 3 * 32 + 1, :], den_t[0 : 3 * 32 + 1, :])
        for h in range(HPC):
            p, lo = h // 2, (h % 2) * D
            for ih in range(2):
                rb = ps2.tile(
                    [D, N // 2], F32, name=f"rb{h}_{ih}",
                    tag="sA" if (2 * h + ih) % 2 == 0 else "sB", bufs=1,
                )
                for ib in range(2):
                    off = ih * 1024 + ib * 512
                    nc.tensor.matmul(
                        rb[:, ib * 512 : (ib + 1) * 512],
                        ones_q[32 * h : 32 * h + 1, :],
                        rdq[32 * h : 32 * h + 1, off : off + 512],
                        start=True,
                        stop=True,
                        tile_position=(32 * h, 0),
                    )
                nc.vector.tensor_mul(
                    AT[p][lo : lo + D, ih * 1024 : (ih + 1) * 1024],
                    OTs[h][:, ih * 1024 : (ih + 1) * 1024],
                    rb[:],
                )

        # ---- output projection: Y[n,:] = sum_p AT_p[:,n]^T @ WoT_p (K=128) ----
        for nchunk in range(NJC):
            py = ps2.tile(
                [128, DIM], F32, name=f"py{nchunk}",
                tag="sA" if nchunk % 2 == 0 else "sB", bufs=1,
            )
            for oh in range(2):
                for p in range(2):
                    nc.tensor.matmul(
                        py[:, oh * 512 : (oh + 1) * 512],
                        AT[p][:, nchunk * 128 : (nchunk + 1) * 128],
                        wo_t[p][:, oh * 512 : (oh + 1) * 512],
                        start=(p == 0),
                        stop=(p == 1),
                    )
            yt = ysb.tile([128, DIM], F32, name=f"yt{nchunk}", tag="yt")
            if nchunk % 2 == 0:
                nc.scalar.copy(yt[:], py[:])
            else:
                nc.vector.tensor_copy(yt[:], py[:])
            nc.sync.dma_start(y[nchunk * 128 : (nchunk + 1) * 128, :], yt[:])

    nc.compile()
    return nc


def _get_nc():
    if "nc" not in _CACHE:
        _CACHE["nc"] = _build_nc()
    return _CACHE["nc"]


def kernel(x, tar, mask, W_qv, W_k, W_out, b_out):
    from concourse import bass_utils

    x = np.asarray(x, np.float32)
    tar = np.asarray(tar, np.float32)
    mask = np.asarray(mask).astype(bool)
    W_qv = np.asarray(W_qv, np.float32)
    W_k = np.asarray(W_k, np.float32)
    W_out = np.asarray(W_out, np.float32)
    b_out = np.asarray(b_out, np.float32)

    m_pad = np.concatenate([np.ones((B, 1), bool), mask], axis=1)  # [B, N]
    maskadd_f = np.where(m_pad, 0.0, -30000.0).astype(np.float32)

    nc = _get_nc()
    in_maps = []
    for c in range(NCORES):
        b = c // 4
        h0 = (c % 4) * HPC
        rows = slice(h0 * D, h0 * D + HPC * D)
        vrows = slice(DIM + h0 * D, DIM + h0 * D + HPC * D)
        in_maps.append(
            {
                "xT": np.ascontiguousarray(x[b].T),
                "tarT": np.ascontiguousarray(tar[b].T),
                "wqT": np.ascontiguousarray((W_qv[rows] * np.float32(0.03125)).T),
                "wvT": np.ascontiguousarray(W_qv[vrows].T),
                "wkT": np.ascontiguousarray(W_k[rows].T),
                "woTp": np.ascontiguousarray(W_out[:, rows].T.reshape(2, 128, DIM)),
                "maskadd": np.ascontiguousarray(maskadd_f[b].reshape(NJC, 128).T),
                "onesc": np.ones((128, HPC), np.float32),
                "onesq": np.ones((128, D), np.float32),
            }
        )

    res = bass_utils.run_bass_kernel_spmd(nc, in_maps, core_ids=list(range(NCORES)))
    out = np.empty((B, N, DIM), np.float32)
    for b in range(B):
        acc = res.results[4 * b]["y"].copy()
        for c in range(4 * b + 1, 4 * b + 4):
            acc += res.results[c]["y"]
        acc += b_out
        acc[~m_pad[b]] = np.nan
        out[b] = acc
    return out


# revision 11
# speedup vs baseline: 1.1249x; 1.1249x over previous
"""Trainium2 Bass kernel for nn_Attention_Dec (dense cross-attention, B=2 N=2048
DIM=1024 H=16 heads of d=64, padding mask, softmax over x-positions).

Sharding: core c handles batch b=c//4 and 4 heads h0=(c%4)*4 (tensor-parallel
over heads within a batch).  Each core computes a partial output
Y_c = (softmax-attn for its 4 heads) @ W_out[:, cols].T  of shape [2048, 1024];
host sums the 4 partials per batch, adds b_out, and writes NaN rows where the
(front-padded) mask is False — matching the reference's all--inf softmax NaNs.

Device-side per core (all matmuls float32r, fp32 PSUM accumulate):
  KT[d,i] = Wk_h @ tar^T          (i = tar positions)
  QT[d,j] = (Wq_h/32) @ x^T       (j = x positions; scale folded into weights)
  V[j,d]  = x @ Wv_h^T            (+ ones column for the softmax denominator)
  S^T[j,i] = QT^T·KT  (per head, K=64 contraction)
  P^T = exp(S^T + maskadd_j)      (per-partition ACT bias masks x positions;
                                   -30000 underflows exp to exactly 0)
  OT[d,i] = sum_j V_aug[j,d]·P^T[j,i]  (row 64 = softmax denominator den_i)
  AT[d,i] = OT[d,i] * (1/den_i)   (batched reciprocal + ones-broadcast matmul)
  Y[n,o] += AT_pair[:,n]^T @ WoT_pair  (K=128, accumulated over head pairs)

PSUM layout: tag "s2" = 2x [128,1024] (4 banks, double-buffered: projections,
S^T, recip-broadcast, final Y) + tag "ot" = 1x [65,2048] (4 banks: V rounds,
then the per-head PV accumulator).
"""

from contextlib import ExitStack

import numpy as np

B, N, DIM, H = 2, 2048, 1024, 16
D = 64
HPC = 4  # heads per core
NCORES = 8
NJC = N // 128  # 16 j-chunks
NKC = DIM // 128  # 8 contraction chunks

_CACHE = {}


def _build_nc():
    import concourse.tile as tile
    from concourse import bacc, mybir

    F32 = mybir.dt.float32
    F32R = mybir.dt.float32r
    EXP = mybir.ActivationFunctionType.Exp

    nc = bacc.Bacc("TRN2", debug=False, num_devices=NCORES)

    xT = nc.dram_tensor("xT", [DIM, N], F32R, kind="ExternalInput").ap()
    tarT = nc.dram_tensor("tarT", [DIM, N], F32R, kind="ExternalInput").ap()
    wqT = nc.dram_tensor("wqT", [DIM, HPC * D], F32R, kind="ExternalInput").ap()
    wvT = nc.dram_tensor("wvT", [DIM, HPC * D], F32R, kind="ExternalInput").ap()
    wkT = nc.dram_tensor("wkT", [DIM, HPC * D], F32R, kind="ExternalInput").ap()
    woTp = nc.dram_tensor("woTp", [2, 128, DIM], F32R, kind="ExternalInput").ap()
    maskadd = nc.dram_tensor("maskadd", [128, NJC], F32, kind="ExternalInput").ap()
    onesc = nc.dram_tensor("onesc", [128, HPC], F32R, kind="ExternalInput").ap()
    onesq = nc.dram_tensor("onesq", [128, D], F32R, kind="ExternalInput").ap()
    y = nc.dram_tensor("y", [N, DIM], F32, kind="ExternalOutput").ap()

    with tile.TileContext(nc) as tc, ExitStack() as ctx:
        consts = ctx.enter_context(tc.tile_pool(name="consts", bufs=1))
        wpool = ctx.enter_context(tc.tile_pool(name="wpool", bufs=1))
        qkv = ctx.enter_context(tc.tile_pool(name="qkv", bufs=1))
        ps2 = ctx.enter_context(tc.tile_pool(name="ps2", bufs=2, space="PSUM"))
        pot = ctx.enter_context(tc.tile_pool(name="pot", bufs=1, space="PSUM"))

        mk = consts.tile([128, NJC], F32, name="mk")
        nc.sync.dma_start(mk[:], maskadd[:])
        ones_c = consts.tile([128, HPC], F32R, name="ones_c")
        nc.sync.dma_start(ones_c[:], onesc[:])
        ones_q = consts.tile([128, D], F32R, name="ones_q")
        nc.sync.dma_start(ones_q[:], onesq[:])

        wq_t, wv_t, wk_t = [], [], []
        for kc in range(NKC):
            t = wpool.tile([128, HPC * D], F32R, name=f"wq{kc}", tag=f"wq{kc}")
            nc.sync.dma_start(t[:], wqT[kc * 128 : (kc + 1) * 128, :])
            wq_t.append(t)
            t = wpool.tile([128, HPC * D], F32R, name=f"wv{kc}", tag=f"wv{kc}")
            nc.sync.dma_start(t[:], wvT[kc * 128 : (kc + 1) * 128, :])
            wv_t.append(t)
            t = wpool.tile([128, HPC * D], F32R, name=f"wk{kc}", tag=f"wk{kc}")
            nc.sync.dma_start(t[:], wkT[kc * 128 : (kc + 1) * 128, :])
            wk_t.append(t)

        KT = [qkv.tile([128, N], F32R, name=f"KT{p}", tag=f"KT{p}") for p in range(2)]
        QT = [qkv.tile([128, N], F32R, name=f"QT{p}", tag=f"QT{p}") for p in range(2)]
        VA = [
            qkv.tile([128, HPC, D + 1], F32R, name=f"VA{jc}", tag=f"VA{jc}")
            for jc in range(NJC)
        ]

        # ---- projections: KT from tarT (pool released), then QT+V from xT ----
        with tc.tile_pool(name="tart", bufs=1) as tartp:
            tt = []
            for kc in range(NKC):
                t = tartp.tile([128, N], F32R, name=f"tart{kc}", tag=f"tart{kc}")
                nc.sync.dma_start(t[:], tarT[kc * 128 : (kc + 1) * 128, :])
                tt.append(t)
            psk = [
                (ps2 if 2 * p + hf < 2 else pot).tile(
                    [128, N // 2], F32, name=f"psk{p}{hf}",
                    tag=["sA", "sB", "otA", "otB"][2 * p + hf], bufs=1,
                )
                for p in range(2)
                for hf in range(2)
            ]
            for kc in range(NKC):
                for p in range(2):
                    for hf in range(2):
                        for ib in range(2):
                            off = hf * 1024 + ib * 512
                            nc.tensor.matmul(
                                psk[2 * p + hf][:, ib * 512 : (ib + 1) * 512],
                                wk_t[kc][:, p * 128 : (p + 1) * 128],
                                tt[kc][:, off : off + 512],
                                start=(kc == 0),
                                stop=(kc == NKC - 1),
                            )
            for p in range(2):
                for hf in range(2):
                    nc.scalar.copy(
                        KT[p][:, hf * 1024 : (hf + 1) * 1024], psk[2 * p + hf][:]
                    )

        with tc.tile_pool(name="xt", bufs=1) as xtp:
            xt = []
            for kc in range(NKC):
                t = xtp.tile([128, N], F32R, name=f"xt{kc}", tag=f"xt{kc}")
                nc.sync.dma_start(t[:], xT[kc * 128 : (kc + 1) * 128, :])
                xt.append(t)
            psq = [
                (ps2 if 2 * p + hf < 2 else pot).tile(
                    [128, N // 2], F32, name=f"psq{p}{hf}",
                    tag=["sA", "sB", "otA", "otB"][2 * p + hf], bufs=1,
                )
                for p in range(2)
                for hf in range(2)
            ]
            for kc in range(NKC):
                for p in range(2):
                    for hf in range(2):
                        for ib in range(2):
                            off = hf * 1024 + ib * 512
                            nc.tensor.matmul(
                                psq[2 * p + hf][:, ib * 512 : (ib + 1) * 512],
                                wq_t[kc][:, p * 128 : (p + 1) * 128],
                                xt[kc][:, off : off + 512],
                                start=(kc == 0),
                                stop=(kc == NKC - 1),
                            )
            for p in range(2):
                for hf in range(2):
                    nc.scalar.copy(
                        QT[p][:, hf * 1024 : (hf + 1) * 1024], psq[2 * p + hf][:]
                    )
            for jc in range(NJC):
                pv = ps2.tile(
                    [128, HPC * D], F32, name=f"pv{jc}",
                    tag="sA" if jc % 2 == 0 else "sB", bufs=1,
                )
                for kc in range(NKC):
                    nc.tensor.matmul(
                        pv[:],
                        xt[kc][:, jc * 128 : (jc + 1) * 128],
                        wv_t[kc][:],
                        start=(kc == 0),
                        stop=(kc == NKC - 1),
                    )
                nc.vector.tensor_copy(VA[jc][:, :, 0:D], pv[:])
                nc.vector.tensor_copy(VA[jc][:, :, D], ones_c[:])

        # ---- late pool (reuses released tart/xt space) ----
        late = ctx.enter_context(tc.tile_pool(name="late", bufs=1))
        wo_t = []
        for p in range(2):
            t = late.tile([128, DIM], F32R, name=f"wo{p}", tag=f"wo{p}")
            nc.sync.dma_start(t[:], woTp[p])
            wo_t.append(t)
        OTs = [
            late.tile([D + 1, N], F32, name=f"OTs{h}", tag=f"OTs{h}")
            for h in range(HPC)
        ]
        AT = [late.tile([128, N], F32R, name=f"AT{p}", tag=f"AT{p}") for p in range(2)]
        den_t = late.tile([128, N], F32, name="den_t")
        rdq = late.tile([128, N], F32R, name="rdq")
        ptp = ctx.enter_context(tc.tile_pool(name="ptp", bufs=4))
        ysb = ctx.enter_context(tc.tile_pool(name="ysb", bufs=2))

        # ---- attention: head pairs row-tiled on the PE (A at rows 0-63,
        # B at rows 64-127), i-halves outer so S_A+S_B+OT_A+OT_B = 8 banks ----
        for p in range(2):
            hA, hB = 2 * p, 2 * p + 1
            for ih in range(2):
                ioff = ih * 1024
                otA = pot.tile([D + 1, N // 2], F32, name=f"otA{p}_{ih}", tag="otA")
                otB = pot.tile([D + 1, N // 2], F32, name=f"otB{p}_{ih}", tag="otB")
                for jc in range(NJC):
                    stA = ps2.tile(
                        [128, N // 2], F32, name=f"stA{p}_{jc}_{ih}", tag="sA",
                        bufs=1,
                    )
                    stB = ps2.tile(
                        [128, N // 2], F32, name=f"stB{p}_{jc}_{ih}", tag="sB",
                        bufs=1,
                    )
                    for ib in range(2):
                        off = ioff + ib * 512
                        nc.tensor.matmul(
                            stA[:, ib * 512 : (ib + 1) * 512],
                            QT[p][0:D, jc * 128 : (jc + 1) * 128],
                            KT[p][0:D, off : off + 512],
                            start=True,
                            stop=True,
                        )
                        nc.tensor.matmul(
                            stB[:, ib * 512 : (ib + 1) * 512],
                            QT[p][D:128, jc * 128 : (jc + 1) * 128],
                            KT[p][D:128, off : off + 512],
                            start=True,
                            stop=True,
                        )
                    ptA = ptp.tile(
                        [128, N // 2], F32R, name=f"ptA{p}_{jc}_{ih}", tag="pt"
                    )
                    nc.scalar.activation(
                        ptA[:], stA[:], EXP, bias=mk[:, jc : jc + 1], scale=1.0
                    )
                    ptB = ptp.tile(
                        [128, N // 2], F32R, name=f"ptB{p}_{jc}_{ih}", tag="pt"
                    )
                    nc.scalar.activation(
                        ptB[:], stB[:], EXP, bias=mk[:, jc : jc + 1], scale=1.0
                    )
                    for ib in range(2):
                        nc.tensor.matmul(
                            otA[:, ib * 512 : (ib + 1) * 512],
                            VA[jc][:, hA, :],
                            ptA[:, ib * 512 : (ib + 1) * 512],
                            start=(jc == 0),
                            stop=(jc == NJC - 1),
                        )
                        nc.tensor.matmul(
                            otB[:, ib * 512 : (ib + 1) * 512],
                            VA[jc][:, hB, :],
                            ptB[:, ib * 512 : (ib + 1) * 512],
                            start=(jc == 0),
                            stop=(jc == NJC - 1),
                        )
                nc.vector.tensor_copy(OTs[hA][:, ioff : ioff + 1024], otA[:])
                nc.vector.tensor_copy(OTs[hB][:, ioff : ioff + 1024], otB[:])

        # ---- normalize: AT[pair] rows = OTs_h * broadcast(1/den_h) ----
        for h in range(HPC):
            nc.scalar.copy(den_t[32 * h : 32 * h + 1, :], OTs[h][D : D + 1, :])
        with nc.allow_low_precision(reason="f32r reciprocal, 1e-4 rel is fine"):
            nc.vector.reciprocal(rdq[0 : 3 * 32 + 1, :], den_t[0 : 3 * 32 + 1, :])
        for h in range(HPC):
            p, lo = h // 2, (h % 2) * D
            for ih in range(2):
                rb = ps2.tile(
                    [D, N // 2], F32, name=f"rb{h}_{ih}",
                    tag="sA" if (2 * h + ih) % 2 == 0 else "sB", bufs=1,
                )
                for ib in range(2):
                    off = ih * 1024 + ib * 512
                    nc.tensor.matmul(
                        rb[:, ib * 512 : (ib + 1) * 512],
                        ones_q[32 * h : 32 * h + 1, :],
                        rdq[32 * h : 32 * h + 1, off : off + 512],
                        start=True,
                        stop=True,
                        tile_position=(32 * h, 0),
                    )
                nc.vector.tensor_mul(
                    AT[p][lo : lo + D, ih * 1024 : (ih + 1) * 1024],
                    OTs[h][0:D, ih * 1024 : (ih + 1) * 1024],
                    rb[:],
                )

        # ---- output projection: Y[n,:] = sum_p AT_p[:,n]^T @ WoT_p (K=128) ----
        for nchunk in range(NJC):
            py = ps2.tile(
                [128, DIM], F32, name=f"py{nchunk}",
                tag="sA" if nchunk % 2 == 0 else "sB", bufs=1,
            )
            for oh in range(2):
                for p in range(2):
                    nc.tensor.matmul(
                        py[:, oh * 512 : (oh + 1) * 512],
                        AT[p][:, nchunk * 128 : (nchunk + 1) * 128],
                        wo_t[p][:, oh * 512 : (oh + 1) * 512],
                        start=(p == 0),
                        stop=(p == 1),
                    )
            yt = ysb.tile([128, DIM], F32, name=f"yt{nchunk}", tag="yt")
            if nchunk % 2 == 0:
                nc.scalar.copy(yt[:], py[:])
            else:
                nc.vector.tensor_copy(yt[:], py[:])
            nc.sync.dma_start(y[nchunk * 128 : (nchunk + 1) * 128, :], yt[:])

    nc.compile()
    return nc


def _get_nc():
    if "nc" not in _CACHE:
        _CACHE["nc"] = _build_nc()
    return _CACHE["nc"]


def kernel(x, tar, mask, W_qv, W_k, W_out, b_out):
    from concourse import bass_utils

    x = np.asarray(x, np.float32)
    tar = np.asarray(tar, np.float32)
    mask = np.asarray(mask).astype(bool)
    W_qv = np.asarray(W_qv, np.float32)
    W_k = np.asarray(W_k, np.float32)
    W_out = np.asarray(W_out, np.float32)
    b_out = np.asarray(b_out, np.float32)

    m_pad = np.concatenate([np.ones((B, 1), bool), mask], axis=1)  # [B, N]
    maskadd_f = np.where(m_pad, 0.0, -30000.0).astype(np.float32)

    nc = _get_nc()
    in_maps = []
    for c in range(NCORES):
        b = c // 4
        h0 = (c % 4) * HPC
        rows = slice(h0 * D, h0 * D + HPC * D)
        vrows = slice(DIM + h0 * D, DIM + h0 * D + HPC * D)
        in_maps.append(
            {
                "xT": np.ascontiguousarray(x[b].T),
                "tarT": np.ascontiguousarray(tar[b].T),
                "wqT": np.ascontiguousarray((W_qv[rows] * np.float32(0.03125)).T),
                "wvT": np.ascontiguousarray(W_qv[vrows].T),
                "wkT": np.ascontiguousarray(W_k[rows].T),
                "woTp": np.ascontiguousarray(W_out[:, rows].T.reshape(2, 128, DIM)),
                "maskadd": np.ascontiguousarray(maskadd_f[b].reshape(NJC, 128).T),
                "onesc": np.ones((128, HPC), np.float32),
                "onesq": np.ones((128, D), np.float32),
            }
        )

    res = bass_utils.run_bass_kernel_spmd(nc, in_maps, core_ids=list(range(NCORES)))
    out = np.empty((B, N, DIM), np.float32)
    for b in range(B):
        acc = res.results[4 * b]["y"].copy()
        for c in range(4 * b + 1, 4 * b + 4):
            acc += res.results[c]["y"]
        acc += b_out
        acc[~m_pad[b]] = np.nan
        out[b] = acc
    return out
